# revision 1
# baseline (speedup 1.0000x reference)
"""Trainium2 Bass kernel for a CIF (continuous-integrate-and-fire) layer.

Takes FULL inputs (B=16), shards batch-parallel across 8 NeuronCores
(2 batch items per core), runs one Bass/Tile program per core via
run_bass_kernel_spmd, and gathers the full (16, 512, 512) output.

Math: the CIF scatter is reformulated as interval overlap,
  A[s,t] = clamp(csum[s]-t,0,1) - clamp(csum[s-1]-t,0,1)
which telescopes into
  out[t] = scale*(Ru[s2-1]-Ru[s1-1]) + (1+t-c[s2-1])*x[s2] + (c[s1-1]-t)*x[s1]
with Ru = prefix-sum of alpha_u * x (unscaled), c = scale*csum_u,
s1 = first s with scale*csum_u[s] > t, s2 = first s with scale*csum_u[s] >= t+1.
This is exact as long as every step fires at most once (alpha <= 1 after
scaling), which holds by construction here (scale ~ target_len/alpha_sum << 1).
"""

import os
import numpy as np

BUILD_STAGE = int(os.environ.get("BUILD_STAGE", "5"))
SUB = int(os.environ.get("SUB", "9"))

try:
    import concourse.bass as bass
except ImportError:
    import sys
    sys.path.insert(0, "/opt/trn_rl_repo")
    import concourse.bass as bass

import concourse.tile as tile
from concourse import bacc, mybir
from concourse.bass_utils import run_bass_kernel_spmd
from concourse.masks import make_identity, make_upper_triangular

F32 = mybir.dt.float32
F32R = mybir.dt.float32r
I32 = mybir.dt.int32
AF = mybir.ActivationFunctionType
OP = mybir.AluOpType

B, S, C, T = 16, 4096, 512, 512
NCORES = 8
BL = B // NCORES          # batch items per core
NBLK = S // 128           # 32 s-blocks per batch item
NT = T // 128             # 4 t-tiles
CIF_EPS = 1e-4
LN_EPS = 1e-5


def r(ap):
    """bitcast an fp32 AP to float32r for full-rate PE matmul"""
    return ap.bitcast(F32R)


def build_program():
    nc = bacc.Bacc("TRN2", target_bir_lowering=False, debug=False)

    x_d = nc.dram_tensor("x", [BL, S, C], F32, kind="ExternalInput").ap()
    pad_d = nc.dram_tensor("encoder_padding_mask", [BL, S], mybir.dt.uint8,
                           kind="ExternalInput").ap()
    tl_d = nc.dram_tensor("target_lengths", [BL], I32, kind="ExternalInput").ap()
    convw_d = nc.dram_tensor("conv_w", [C, C, 3], F32, kind="ExternalInput").ap()
    convb_d = nc.dram_tensor("conv_b", [C], F32, kind="ExternalInput").ap()
    lng_d = nc.dram_tensor("ln_g", [C], F32, kind="ExternalInput").ap()
    lnb_d = nc.dram_tensor("ln_b", [C], F32, kind="ExternalInput").ap()
    projw_d = nc.dram_tensor("proj_w", [C, 1], F32, kind="ExternalInput").ap()
    projb_d = nc.dram_tensor("proj_b", [1], F32, kind="ExternalInput").ap()
    out_d = nc.dram_tensor("out", [BL, T, C], F32, kind="ExternalOutput").ap()
    dbg = {}
    if os.environ.get("KDEBUG") == "1":
        dbg["alpha"] = nc.dram_tensor("dbg_alpha", [BL, 128, NBLK], F32,
                                      kind="ExternalOutput").ap()
        dbg["csum"] = nc.dram_tensor("dbg_csum", [BL, 32, 128], F32,
                                     kind="ExternalOutput").ap()
        dbg["sidx"] = nc.dram_tensor("dbg_sidx", [BL, 2, 128, NT], mybir.dt.int32,
                                     kind="ExternalOutput").ap()
        dbg["cprev"] = nc.dram_tensor("dbg_cprev", [BL, 2, 128, NT], F32,
                                      kind="ExternalOutput").ap()
        dbg["cols"] = nc.dram_tensor("dbg_cols", [BL, 128, 8], F32,
                                     kind="ExternalOutput").ap()
        dbg["gr"] = nc.dram_tensor("dbg_gr", [BL, 2, 128, NT, C], F32,
                                   kind="ExternalOutput").ap()
        dbg["gx"] = nc.dram_tensor("dbg_gx", [BL, 2, 128, NT, C], F32,
                                   kind="ExternalOutput").ap()
        dbg["oa"] = nc.dram_tensor("dbg_oa", [BL, 2, NT, 128, C], F32,
                                   kind="ExternalOutput").ap()

    with tile.TileContext(nc) as tc:
        with (
            tc.tile_pool(name="const", bufs=1) as cpool,
            tc.tile_pool(name="work", bufs=2) as wpool,
            tc.tile_pool(name="ps", bufs=2, space="PSUM") as pspool,
            tc.tile_pool(name="dram", bufs=1, space="DRAM") as dpool,
        ):
            build_kernel(nc, tc, cpool, wpool, pspool, dpool,
                         x_d, pad_d, tl_d, convw_d, convb_d, lng_d, lnb_d,
                         projw_d, projb_d, out_d, dbg)
    nc.compile()
    return nc


def build_kernel(nc, tc, cpool, wpool, pspool, dpool,
                 x_d, pad_d, tl_d, convw_d, convb_d, lng_d, lnb_d,
                 projw_d, projb_d, out_d, dbg={}):
    # ---------------- constants ----------------
    ident = cpool.tile([128, 128], F32)
    make_identity(nc, ident[:])
    ident_r = cpool.tile([128, 128], F32R)
    nc.scalar.copy(ident_r[:], ident[:])
    ut128 = cpool.tile([128, 128], F32)        # ut[k,m] = 1{k<=m}
    make_upper_triangular(nc, ut128[:], 1.0, diag=True)
    su32 = cpool.tile([32, 32], F32)           # su[k,m] = 1{k<m}
    make_upper_triangular(nc, su32[:], 1.0, diag=False)
    ones_1x128 = cpool.tile([1, 128], F32)
    nc.gpsimd.memset(ones_1x128[:], 1.0)
    ones_r = cpool.tile([1, 128], F32R)
    nc.scalar.copy(ones_r[:], ones_1x128[:])

    iota_i = cpool.tile([128, 1], I32)
    nc.gpsimd.iota(iota_i[:], pattern=[[0, 1]], base=0, channel_multiplier=1)
    iota_col = cpool.tile([128, 1], F32)       # iota_col[p] = p
    nc.vector.tensor_copy(iota_col[:], iota_i[:])
    iota32_i = cpool.tile([128, 32], I32)
    nc.gpsimd.iota(iota32_i[:], pattern=[[1, 32]], base=0, channel_multiplier=0)
    iota32_rep = cpool.tile([128, 32], F32)    # iota32_rep[p, j] = j
    nc.vector.tensor_copy(iota32_rep[:], iota32_i[:])

    zeros_32x128 = cpool.tile([32, 128], F32)
    nc.gpsimd.memset(zeros_32x128[:], 0.0)
    zrow = cpool.tile([1, C], F32)
    nc.gpsimd.memset(zrow[:], 0.0)

    # ---------------- load + replicate small params ----------------
    convb_row = cpool.tile([1, C], F32)
    nc.sync.dma_start(convb_row[:], convb_d[:].rearrange("(a c) -> a c", a=1))
    convb_r = cpool.tile([1, C], F32R)
    nc.scalar.copy(convb_r[:], convb_row[:])
    lng_row = cpool.tile([1, C], F32)
    nc.sync.dma_start(lng_row[:], lng_d[:].rearrange("(a c) -> a c", a=1))
    lnb_row = cpool.tile([1, C], F32)
    nc.sync.dma_start(lnb_row[:], lnb_d[:].rearrange("(a c) -> a c", a=1))
    pw_row = cpool.tile([1, C], F32)
    nc.sync.dma_start(pw_row[:], projw_d[:].rearrange("c 1 -> 1 c"))
    pb_row = cpool.tile([1, 1], F32)
    nc.sync.dma_start(pb_row[:], projb_d[:].rearrange("(a c) -> a c", a=1))
    tl_sb = cpool.tile([1, BL], I32)
    nc.sync.dma_start(tl_sb[:], tl_d[:].rearrange("(a b) -> a b", a=1))

    def replicate(row_ap, n, nm):
        """(1, n) -> (128, n) via K=1 ones matmul"""
        ps = pspool.tile([128, n], F32, tag="pss", name=nm, bufs=3)
        nc.tensor.matmul(ps[:], lhsT=ones_1x128[:], rhs=row_ap, start=True,
                         stop=True)
        return ps

    g_rep = cpool.tile([128, C], F32)
    nc.scalar.copy(g_rep[:], replicate(lng_row[:], C, "repg")[:])
    b_rep = cpool.tile([128, C], F32)
    nc.scalar.copy(b_rep[:], replicate(lnb_row[:], C, "repb")[:])
    pw_rep = cpool.tile([128, C], F32)
    nc.scalar.copy(pw_rep[:], replicate(pw_row[:], C, "reppw")[:])
    pb_col = cpool.tile([128, 1], F32)
    nc.scalar.copy(pb_col[:], replicate(pb_row[:], 1, "reppb")[:])

    # ---------------- conv weights: native load + PE transpose -> (ci, co) --
    # wt[:, k*4+q, :] = conv_w[:, 128q:128(q+1), k].T   (ci=128 part, co=512)
    wt = cpool.tile([128, 12, C], F32R)
    for cchunk in range(4):
        wnat = wpool.tile([128, C, 3], F32, tag="wnat", bufs=1)
        nc.sync.dma_start(wnat[:], convw_d[128 * cchunk:128 * (cchunk + 1), :, :])
        for k in range(3):
            pst = pspool.tile([128, 512], F32, tag="h", name="pst", bufs=3)
            for q in range(4):
                nc.tensor.transpose(
                    out=pst[:, 128 * q:128 * (q + 1)],
                    in_=wnat[:, 128 * q:128 * (q + 1), k],
                    identity=ident[:],
                )
            for q in range(4):
                nc.scalar.copy(
                    wt[:, k * 4 + q, 128 * cchunk:128 * (cchunk + 1)],
                    pst[:, 128 * q:128 * (q + 1)],
                )

    R_dram = [dpool.tile([S + 1, C], F32, tag=f"Rd{b}", name=f"Rd{b}")
              for b in range(BL)]

    for b in range(BL):
        # zero row 0 of R_dram
        nc.sync.dma_start(R_dram[b][0:1, :], zrow[:])

        # padding mask -> (128, 32) f32, inverted
        padu8 = wpool.tile([128, NBLK], mybir.dt.uint8, tag="padu8")
        nc.sync.dma_start(padu8[:], pad_d[b].rearrange("(i p) -> p i", p=128))
        invpad = wpool.tile([128, NBLK], F32, tag="invpad")
        padf = wpool.tile([128, NBLK], F32, tag="padf")
        nc.vector.tensor_copy(padf[:], padu8[:])
        nc.vector.tensor_scalar(invpad[:], padf[:], -1.0, 1.0, OP.mult, OP.add)

        alpha_sb = wpool.tile([128, NBLK], F32, tag="alpha",
                              name=f"alpha{b}")
        bs_sb = wpool.tile([32, C], F32, tag="bssb", name=f"bssb{b}")

        # ---------------- phase A: predictor + R partial scan ----------------
        # xtw[i]: transposed x window; cols 0..129 = x rows 128i-1 .. 128i+128
        # one-block lag: transpose block ii, then run the predictor for block
        # ii-1 (whose window is complete only after block ii's transpose).
        xtw = [None] * NBLK
        xins = [None] * NBLK
        for ii in range(NBLK + 1):
          if ii < NBLK:
            i = ii
            xt_in = wpool.tile([128, C], F32R, tag="xin", bufs=4,
                               name=f"xin{b}_{i}")
            xins[i] = xt_in
            nc.sync.dma_start(xt_in[:],
                              x_d[b, 128 * i:128 * (i + 1), :].bitcast(F32R))

            xtw[i] = wpool.tile([128, 4, 130], F32R, tag="xtw", bufs=4,
                                name=f"xtw{b}_{i}")
            ps_xt = pspool.tile([128, 512], F32R, tag="xt", name="ps_xt", bufs=1)
            for q in range(4):
                nc.tensor.transpose(
                    out=ps_xt[:, 128 * q:128 * (q + 1)],
                    in_=xt_in[:, 128 * q:128 * (q + 1)],
                    identity=ident_r[:],
                )
            ps_xt_v = ps_xt[:].rearrange("p (q s) -> p q s", q=4)
            nc.scalar.copy(xtw[i][:, :, 1:129], ps_xt_v)
            if i == 0:
                nc.vector.tensor_scalar_mul(
                    xtw[0][:, :, 0:1],
                    ident_r[:, 0:4].rearrange("p (a o) -> p a o", o=1), 0.0)
            else:
                # col 0 of window i = x row 128i-1 = col 128 of window i-1
                nc.vector.tensor_copy(xtw[i][:, :, 0:1],
                                      xtw[i - 1][:, :, 128:129])
                # col 129 of window i-1 = x row 128i = this block's first col
                nc.vector.tensor_copy(xtw[i - 1][:, :, 129:130],
                                      ps_xt_v[:, :, 0:1])
            if i == NBLK - 1:
                nc.vector.tensor_scalar_mul(
                    xtw[i][:, :, 129:130],
                    ident_r[:, 0:4].rearrange("p (a o) -> p a o", o=1), 0.0)

          if ii >= 1:
            i = ii - 1
            xt_in = xins[i]
            # conv: h[s, co] = sum_k sum_ci x[s+k-1, ci] W[co, ci, k] + conv_b
            ps_h = pspool.tile([128, C], F32, tag="h", name="ps_h", bufs=3)
            first = True
            for k in range(3):
                for q in range(4):
                    nc.tensor.matmul(
                        ps_h[:],
                        lhsT=xtw[i][:, q, k:k + 128],
                        rhs=wt[:, k * 4 + q, :],
                        start=first, stop=False,
                    )
                    first = False
            nc.tensor.matmul(ps_h[:], lhsT=ones_r[:], rhs=convb_r[:],
                             start=False, stop=True)

            # layernorm
            if SUB < 1:
                continue
            st6 = wpool.tile([128, 6], F32, tag="st6", bufs=3)
            nc.vector.bn_stats(st6[:], ps_h[:])
            mv = wpool.tile([128, 2], F32, tag="mv", bufs=3)
            nc.vector.bn_aggr(mv[:], st6[:])
            vpe = wpool.tile([128, 1], F32, tag="vpe")
            nc.vector.tensor_scalar_add(vpe[:], mv[:, 1:2], LN_EPS)
            stdv = wpool.tile([128, 1], F32, tag="stdv")
            nc.scalar.sqrt(stdv[:], vpe[:])
            rstd = wpool.tile([128, 1], F32, tag="rstd")
            nc.vector.reciprocal(rstd[:], stdv[:])
            z = wpool.tile([128, C], F32, tag="z", bufs=3)
            nc.vector.tensor_scalar(z[:], ps_h[:], mv[:, 0:1], rstd[:, 0:1],
                                    OP.subtract, OP.mult)
            u = wpool.tile([128, C], F32, tag="u", bufs=3)
            nc.gpsimd.tensor_mul(u[:], z[:], g_rep[:])
            u2 = wpool.tile([128, C], F32, tag="u2", bufs=3)
            nc.gpsimd.tensor_add(u2[:], u[:], b_rep[:])
            gel = wpool.tile([128, C], F32, tag="gel", bufs=3)
            nc.scalar.activation(gel[:], u2[:], AF.Gelu)
            # projection: logit = sum_c gel*pw  (+pb inside sigmoid)
            scr = wpool.tile([128, C], F32, tag="scr", bufs=2)
            logit = wpool.tile([128, 1], F32, tag="logit")
            nc.vector.scalar_tensor_tensor(scr[:], gel[:], 1.0, pw_rep[:],
                                           OP.mult, OP.mult,
                                           accum_out=logit[:])
            araw = wpool.tile([128, 1], F32, tag="araw")
            nc.scalar.activation(araw[:], logit[:], AF.Sigmoid,
                                 bias=pb_col[:, 0:1])
            nc.vector.tensor_mul(alpha_sb[:, i:i + 1], araw[:],
                                 invpad[:, i:i + 1])


        # ---------------- R partial scan (decoupled from predictor) -------
        if SUB >= 2:
            for i in range(NBLK):
                xin2 = wpool.tile([128, C], F32R, tag="xin2", bufs=3,
                                  name=f"xin2_{b}_{i}")
                nc.sync.dma_start(
                    xin2[:], x_d[b, 128 * i:128 * (i + 1), :].bitcast(F32R))
                uta = wpool.tile([128, 128], F32R, tag="uta", bufs=3)
                nc.vector.tensor_scalar_mul(uta[:], ut128[:],
                                            alpha_sb[:, i:i + 1])
                ps_rp = pspool.tile([128, C], F32, tag="rp", name="ps_rp",
                                    bufs=1)
                nc.tensor.matmul(ps_rp[:], lhsT=uta[:], rhs=xin2[:],
                                 start=True, stop=True)
                rp_sb = wpool.tile([128, C], F32, tag="rpsb", bufs=3)
                nc.vector.tensor_copy(rp_sb[:], ps_rp[:])
                nc.sync.dma_start(R_dram[b][1 + 128 * i:1 + 128 * (i + 1), :],
                                  rp_sb[:])
                nc.sync.dma_start(bs_sb[i:i + 1, :], rp_sb[127:128, :])

        if BUILD_STAGE < 2:
            continue
        # ---------------- block offsets for R ----------------
        ps_off = pspool.tile([32, C], F32, tag="pss", name="ps_off", bufs=3)
        nc.tensor.matmul(ps_off[:], lhsT=su32[:], rhs=bs_sb[:],
                         start=True, stop=True)
        offs_sb = wpool.tile([32, C], F32, tag="offsb")   # exclusive offsets
        nc.scalar.copy(offs_sb[:], ps_off[:])

        # ---------------- csum of alpha (unscaled) ----------------
        ps_at = pspool.tile([32, 128], F32, tag="pss", name="ps_at", bufs=3)
        nc.tensor.transpose(out=ps_at[:], in_=alpha_sb[:], identity=ident[:])
        aT = wpool.tile([32, 128], F32, tag="aT")
        nc.scalar.copy(aT[:], ps_at[:])
        csum_u = wpool.tile([32, 128], F32, tag="csumu")
        nc.vector.tensor_tensor_scan(csum_u[:], zeros_32x128[:], aT[:], 0.0,
                                     OP.add, OP.add)
        btot = wpool.tile([32, 1], F32, tag="btot")
        nc.vector.tensor_copy(btot[:], csum_u[:, 127:128])
        ps_bo = pspool.tile([32, 1], F32, tag="pss", name="ps_bo", bufs=3)
        nc.tensor.matmul(ps_bo[:], lhsT=su32[:], rhs=btot[:],
                         start=True, stop=True)
        boff = wpool.tile([32, 1], F32, tag="boff")
        nc.scalar.copy(boff[:], ps_bo[:])
        nc.vector.tensor_scalar_add(csum_u[:], csum_u[:], boff[:, 0:1])

        if dbg:
            nc.sync.dma_start(dbg["alpha"][b], alpha_sb[:])
            nc.sync.dma_start(dbg["csum"][b], csum_u[:])
        bend = wpool.tile([32, 1], F32, tag="bend")       # block-end csums
        nc.vector.tensor_copy(bend[:], csum_u[:, 127:128])
        bshift = wpool.tile([32, 1], F32, tag="bshift")   # bend shifted down 1
        nc.vector.memzero(bshift[0:1, :])
        nc.sync.dma_start(bshift[1:32, :], bend[0:31, :])

        # replicate bend / bshift across partitions: (32,1)->(1,32)->(128,32)
        def rep32(col_ap, tag):
            pst = pspool.tile([32, 32], F32, tag="pss", name="rep32t", bufs=3)
            nc.tensor.transpose(out=pst[0:1, 0:32], in_=col_ap,
                                identity=ident[0:32, 0:32])
            row = wpool.tile([1, 32], F32, tag=tag + "row", name=tag + "row")
            nc.scalar.copy(row[:], pst[0:1, 0:32])
            ps = pspool.tile([128, 32], F32, tag="pss", name="rep32m", bufs=3)
            nc.tensor.matmul(ps[:], lhsT=ones_1x128[:], rhs=row[:],
                             start=True, stop=True)
            out = wpool.tile([128, 32], F32, tag=tag, name=tag)
            nc.scalar.copy(out[:], ps[:])
            return out

        bend_rep = rep32(bend[:], "bendrep")
        bshift_rep = rep32(bshift[:], "bshiftrep")

        # ---------------- per-batch scalars ----------------
        sc = wpool.tile([1, 8], F32, tag="scal")
        nc.sync.dma_start(sc[:, 0:1], csum_u[31:32, 127:128])         # asum
        lf = wpool.tile([1, 1], F32, tag="lf")
        nc.vector.tensor_copy(lf[:], tl_sb[:, b:b + 1])               # L as f32
        nc.vector.tensor_scalar_add(sc[:, 1:2], lf[:], CIF_EPS)      # desired
        nc.vector.reciprocal(sc[:, 2:3], sc[:, 0:1])                  # 1/asum
        nc.vector.tensor_mul(sc[:, 3:4], sc[:, 1:2], sc[:, 2:3])      # scale
        nc.vector.reciprocal(sc[:, 4:5], sc[:, 1:2])                  # 1/desired
        nc.vector.tensor_mul(sc[:, 5:6], sc[:, 0:1], sc[:, 4:5])      # inv_scale
        nc.vector.tensor_scalar_mul(sc[:, 6:7], sc[:, 3:4], -1.0)     # -scale
        nc.vector.tensor_copy(sc[:, 7:8], lf[:])                      # L
        ps_sc = pspool.tile([128, 8], F32, tag="pss", name="ps_sc", bufs=3)
        nc.tensor.matmul(ps_sc[:], lhsT=ones_1x128[:], rhs=sc[:],
                         start=True, stop=True)
        cols = wpool.tile([128, 8], F32, tag="cols")
        nc.scalar.copy(cols[:], ps_sc[:])
        if dbg:
            nc.sync.dma_start(dbg["cols"][b], cols[:])
        scale_c = cols[:, 3:4]
        invscale_c = cols[:, 5:6]
        negscale_c = cols[:, 6:7]
        L_c = cols[:, 7:8]

        # ---------------- searchsorted s1/s2 per t-tile ----------------
        # kind 0 (s1): count csum_u <= t*inv_scale       (op is_le)
        # kind 1 (s2): count csum_u <  (t+1)*inv_scale   (op is_lt)
        idxR = [wpool.tile([128, NT], I32, tag=f"idxR{kk}", name=f"idxR{kk}")
                for kk in range(2)]
        idxX = [wpool.tile([128, NT], I32, tag=f"idxX{kk}", name=f"idxX{kk}")
                for kk in range(2)]
        cprev = [wpool.tile([128, NT], F32, tag=f"cprev{kk}", name=f"cprev{kk}")
                 for kk in range(2)]
        offat_sb = [[None] * NT for _ in range(2)]

        if BUILD_STAGE < 3:
            continue
        for kk, cmp_op in ((0, OP.is_le), (1, OP.is_lt)):
            for j in range(NT):
                tau = wpool.tile([128, 1], F32, tag="tau")
                shift = float(128 * j + kk)   # kind1 threshold is t+1
                nc.vector.tensor_scalar(tau[:], iota_col[:], shift,
                                        invscale_c, OP.add, OP.mult)
                # L1: which block
                cmp1 = wpool.tile([128, 32], F32, tag="cmp1")
                bcnt = wpool.tile([128, 1], F32, tag="bcnt")
                nc.vector.tensor_scalar(cmp1[:], bend_rep[:], tau[:, 0:1], 0.0,
                                        cmp_op, OP.add, accum_out=bcnt[:])
                oh1 = wpool.tile([128, 32], F32, tag="oh1")
                nc.vector.tensor_scalar(oh1[:, 0:1], cmp1[:, 0:1], -1.0, 1.0,
                                        OP.mult, OP.add)
                nc.vector.tensor_sub(oh1[:, 1:32], cmp1[:, 0:31], cmp1[:, 1:32])
                # select the straddled block's 128 csum values
                ps_t = pspool.tile([32, 128], F32, tag="pss", name="ps_t",
                                   bufs=3)
                nc.tensor.transpose(out=ps_t[:], in_=oh1[:], identity=ident[:])
                oh1T = wpool.tile([32, 128], F32, tag="oh1T")
                nc.scalar.copy(oh1T[:], ps_t[:])
                ps_sel = pspool.tile([128, 128], F32, tag="pss", name="ps_sel",
                                     bufs=3)
                nc.tensor.matmul(ps_sel[:], lhsT=oh1T[:], rhs=csum_u[:],
                                 start=True, stop=True)
                # L2: position within block
                cmp2 = wpool.tile([128, 128], F32, tag="cmp2")
                cnt = wpool.tile([128, 1], F32, tag="cnt")
                nc.vector.tensor_scalar(cmp2[:], ps_sel[:], tau[:, 0:1], 0.0,
                                        cmp_op, OP.add, accum_out=cnt[:])
                sidx = wpool.tile([128, 1], F32, tag="sidx")
                nc.vector.tensor_scalar(sidx[:], bcnt[:], 128.0, cnt[:, 0:1],
                                        OP.mult, OP.add)
                # csum_u[s-1]: dot(onehot2, selected), fallback prev block end
                oh2 = wpool.tile([128, 128], F32, tag="oh2")
                nc.vector.tensor_sub(oh2[:, 0:127], cmp2[:, 0:127],
                                     cmp2[:, 1:128])
                nc.vector.tensor_copy(oh2[:, 127:128], cmp2[:, 127:128])
                dsel = wpool.tile([128, 128], F32, tag="dsel", bufs=1)
                cs_at = wpool.tile([128, 1], F32, tag="csat")
                nc.vector.scalar_tensor_tensor(dsel[:], oh2[:], 1.0, ps_sel[:],
                                               OP.mult, OP.mult,
                                               accum_out=cs_at[:])
                dsel2 = wpool.tile([128, 32], F32, tag="dsel2", bufs=1)
                bprev_at = wpool.tile([128, 1], F32, tag="bprevat")
                nc.vector.scalar_tensor_tensor(dsel2[:], oh1[:], 1.0,
                                               bshift_rep[:], OP.mult, OP.mult,
                                               accum_out=bprev_at[:])
                nc.vector.tensor_max(cprev[kk][:, j:j + 1], cs_at[:],
                                     bprev_at[:])
                # gather indices (clamped)
                idr_f = wpool.tile([128, 1], F32, tag="idrf")
                nc.vector.tensor_scalar_min(idr_f[:], sidx[:], float(S))
                nc.vector.tensor_copy(idxR[kk][:, j:j + 1], idr_f[:])
                idx_f = wpool.tile([128, 1], F32, tag="idxf")
                nc.vector.tensor_scalar_min(idx_f[:], sidx[:], float(S - 1))
                nc.vector.tensor_copy(idxX[kk][:, j:j + 1], idx_f[:])
                # R offset row for s-1: block idx = bcnt - 1{cnt==0}
                zc = wpool.tile([128, 1], F32, tag="zc")
                nc.vector.tensor_scalar(zc[:], cnt[:], 0.0, None, OP.is_equal)
                blk = wpool.tile([128, 1], F32, tag="blk")
                nc.vector.tensor_sub(blk[:], bcnt[:], zc[:])
                ohb = wpool.tile([128, 32], F32, tag="ohb")
                nc.vector.tensor_scalar(ohb[:], iota32_rep[:], blk[:, 0:1],
                                        None, OP.is_equal)
                ps_obt = pspool.tile([32, 128], F32, tag="pss", name="ps_obt",
                                     bufs=3)
                nc.tensor.transpose(out=ps_obt[:], in_=ohb[:], identity=ident[:])
                ohbT = wpool.tile([32, 128], F32, tag="ohbT")
                nc.scalar.copy(ohbT[:], ps_obt[:])
                ps_oa = pspool.tile([128, C], F32, tag="pss", name="ps_oa",
                                    bufs=3)
                nc.tensor.matmul(ps_oa[:], lhsT=ohbT[:], rhs=offs_sb[:],
                                 start=True, stop=True)
                oa_sb = wpool.tile([128, C], F32, tag=f"oasb{kk}",
                                   name=f"oasb{kk}_{j}")
                nc.scalar.copy(oa_sb[:], ps_oa[:])
                offat_sb[kk][j] = oa_sb

        if dbg:
            for kk in range(2):
                nc.sync.dma_start(dbg["sidx"][b, kk], idxR[kk][:])
                nc.sync.dma_start(dbg["cprev"][b, kk], cprev[kk][:])
        if BUILD_STAGE < 4:
            continue
        # ---------------- gathers ----------------
        x_flat = x_d.rearrange("b s c -> (b s) c")
        gx = []
        for kk in range(2):
            g = wpool.tile([128, NT, C], F32, tag=f"gx{kk}", name=f"gx{kk}",
                           bufs=1)
            for j in range(NT):
                nc.gpsimd.indirect_dma_start(
                    out=g[:, j, :], out_offset=None, in_=x_flat,
                    in_offset=bass.IndirectOffsetOnAxis(
                        ap=idxX[kk][:, j:j + 1], axis=0),
                    element_offset=b * S * C)
            gx.append(g)
        gr = []
        for kk in range(2):
            g = wpool.tile([128, NT, C], F32, tag=f"gr{kk}", name=f"gr{kk}",
                           bufs=1)
            for j in range(NT):
                nc.gpsimd.indirect_dma_start(
                    out=g[:, j, :], out_offset=None, in_=R_dram[b][:],
                    in_offset=bass.IndirectOffsetOnAxis(
                        ap=idxR[kk][:, j:j + 1], axis=0))
            gr.append(g)

        if BUILD_STAGE < 5:
            continue
        if dbg:
            for kk in range(2):
                nc.sync.dma_start(dbg["gr"][b, kk], gr[kk][:])
                nc.sync.dma_start(dbg["gx"][b, kk], gx[kk][:])
                for j in range(NT):
                    nc.sync.dma_start(dbg["oa"][b, kk, j], offat_sb[kk][j][:])
        # ---------------- combine & write out ----------------
        for j in range(NT):
            tcol = wpool.tile([128, 1], F32, tag="tcol")
            nc.vector.tensor_scalar_add(tcol[:], iota_col[:], float(128 * j))
            valid = wpool.tile([128, 1], F32, tag="valid")
            nc.vector.tensor_scalar(valid[:], tcol[:], L_c, None, OP.is_lt)
            # c1 = (scale*cprev0 - t) * valid
            c1 = wpool.tile([128, 1], F32, tag="c1")
            nc.vector.tensor_scalar(c1[:], cprev[0][:, j:j + 1], scale_c,
                                    tcol[:, 0:1], OP.mult, OP.subtract)
            nc.vector.tensor_mul(c1[:], c1[:], valid[:])
            # c2 = ((t+1) - scale*cprev1) * valid
            c2 = wpool.tile([128, 1], F32, tag="c2")
            t1col = wpool.tile([128, 1], F32, tag="t1col")
            nc.vector.tensor_scalar_add(t1col[:], tcol[:], 1.0)
            nc.vector.tensor_scalar(c2[:], cprev[1][:, j:j + 1], negscale_c,
                                    t1col[:, 0:1], OP.mult, OP.add)
            nc.vector.tensor_mul(c2[:], c2[:], valid[:])
            sv = wpool.tile([128, 1], F32, tag="sv")
            nc.vector.tensor_mul(sv[:], scale_c, valid[:])

            # out = sv*(R2 + off2 - R1 - off1) + c2*x2 + c1*x1
            doff = wpool.tile([128, C], F32, tag="doff")
            nc.vector.tensor_sub(doff[:], offat_sb[1][j][:], offat_sb[0][j][:])
            d = wpool.tile([128, C], F32, tag="d")
            nc.vector.tensor_sub(d[:], gr[1][:, j, :], gr[0][:, j, :])
            nc.vector.tensor_add(d[:], d[:], doff[:])
            t0 = wpool.tile([128, C], F32, tag="t0")
            nc.vector.tensor_scalar_mul(t0[:], gx[0][:, j, :], c1[:, 0:1])
            t1 = wpool.tile([128, C], F32, tag="t1")
            nc.vector.scalar_tensor_tensor(t1[:], gx[1][:, j, :], c2[:, 0:1],
                                           t0[:], OP.mult, OP.add)
            ot = wpool.tile([128, C], F32, tag="ot")
            nc.vector.scalar_tensor_tensor(ot[:], d[:], sv[:, 0:1], t1[:],
                                           OP.mult, OP.add)
            nc.sync.dma_start(out_d[b, 128 * j:128 * (j + 1), :], ot[:])


_prog_cache = {}


def _get_prog():
    if "nc" not in _prog_cache:
        _prog_cache["nc"] = build_program()
    return _prog_cache["nc"]


def kernel(**inputs):
    x = np.asarray(inputs["x"], np.float32)
    pad = np.asarray(inputs["encoder_padding_mask"]).astype(np.uint8)
    tl = np.asarray(inputs["target_lengths"]).astype(np.int32)
    conv_w = np.ascontiguousarray(np.asarray(inputs["conv_w"], np.float32))
    conv_b = np.asarray(inputs["conv_b"], np.float32)
    ln_g = np.asarray(inputs["ln_g"], np.float32)
    ln_b = np.asarray(inputs["ln_b"], np.float32)
    proj_w = np.ascontiguousarray(np.asarray(inputs["proj_w"], np.float32))
    proj_b = np.asarray(inputs["proj_b"], np.float32)

    nc = _get_prog()
    in_maps = []
    for core in range(NCORES):
        lo, hi = core * BL, (core + 1) * BL
        in_maps.append({
            "x": np.ascontiguousarray(x[lo:hi]),
            "encoder_padding_mask": np.ascontiguousarray(pad[lo:hi]),
            "target_lengths": np.ascontiguousarray(tl[lo:hi]),
            "conv_w": conv_w, "conv_b": conv_b,
            "ln_g": ln_g, "ln_b": ln_b,
            "proj_w": proj_w, "proj_b": proj_b,
        })
    res = run_bass_kernel_spmd(nc, in_maps, core_ids=list(range(NCORES)))
    out = np.concatenate([res.results[c]["out"] for c in range(NCORES)], axis=0)
    return out.astype(np.float32)


if __name__ == "__main__":
    import reference as ref
    import jax
    jax.config.update("jax_platforms", "cpu")
    inputs = ref.setup_inputs()
    actual = kernel(**{k: np.asarray(v) for k, v in inputs.items()})
    print("kernel output", actual.shape, actual.dtype)



# revision 7
# speedup vs baseline: 1.0609x; 1.0609x over previous
"""Trainium2 Bass kernel for a CIF (continuous-integrate-and-fire) layer.

Takes FULL inputs (B=16), shards batch-parallel across 8 NeuronCores
(2 batch items per core), runs one Bass/Tile program per core via
run_bass_kernel_spmd, and gathers the full (16, 512, 512) output.

Math: the CIF scatter is reformulated as interval overlap,
  A[s,t] = clamp(csum[s]-t,0,1) - clamp(csum[s-1]-t,0,1)
which telescopes into
  out[t] = scale*(Ru[s2-1]-Ru[s1-1]) + (1+t-c[s2-1])*x[s2] + (c[s1-1]-t)*x[s1]
with Ru = prefix-sum of alpha_u * x (unscaled), c = scale*csum_u,
s1 = first s with scale*csum_u[s] > t, s2 = first s with scale*csum_u[s] >= t+1.
Exact when every step fires at most once (alpha <= 1 after scaling).

Perf notes vs the fp32r baseline:
- x fed as fp16 from the host: conv/scan matmuls run fp16, x windows are
  loaded pre-transposed via the DMA XBAR (no PE transposes / PSUM evac).
- conv weights pre-transposed+cast on the host -> no setup transposes.
- Predictor: conv PSUM -> fused Gelu(scale=rstd, bias=-mu*rstd) when
  ln_g==1, ln_b==0 (true for the reference inputs); sigmoid batched per
  batch item so the scalar engine keeps the Gelu table loaded.
- searchsorted: csum[s-1] and R block offsets fetched by indirect DMA
  gathers (offsets accumulated into the R gather with OOB-skip for the
  "before block 0" case) instead of PE select matmuls.
"""

import os
import numpy as np

try:
    import concourse.bass as bass
except ImportError:
    import sys
    sys.path.insert(0, "/opt/trn_rl_repo")
    import concourse.bass as bass

import concourse.tile as tile
from concourse import bacc, mybir
from concourse.bass_utils import run_bass_kernel_spmd
from concourse.masks import make_identity, make_upper_triangular

F32 = mybir.dt.float32
F16 = mybir.dt.float16
I32 = mybir.dt.int32
AF = mybir.ActivationFunctionType
OP = mybir.AluOpType

B, S, C, T = 16, 4096, 512, 512
NCORES = 8
BL = B // NCORES          # batch items per core
NBLK = S // 128           # 32 s-blocks per batch item
NT = T // 128             # 4 t-tiles
CIF_EPS = 1e-4
LN_EPS = 1e-5


def build_program(g1b0=True, cb0=True, pb0=True, pad0=True):
    nc = bacc.Bacc("TRN2", target_bir_lowering=False, debug=False)

    x_d = nc.dram_tensor("x", [BL, S, C], F16, kind="ExternalInput").ap()
    wt_d = nc.dram_tensor("wt", [128, 12, C], F16, kind="ExternalInput").ap()
    pw_d = nc.dram_tensor("pw_rep", [128, C], F32, kind="ExternalInput").ap()
    ut_d = nc.dram_tensor("ut128", [128, 128], F16, kind="ExternalInput").ap()
    tl_d = nc.dram_tensor("target_lengths", [BL], I32, kind="ExternalInput").ap()
    out_d = nc.dram_tensor("out", [BL, T, C], F32, kind="ExternalOutput").ap()
    convb_d = g_d = b_d = pb_d = pad_d = None
    if not cb0:
        convb_d = nc.dram_tensor("convb16", [1, C], F16, kind="ExternalInput").ap()
    if not g1b0:
        g_d = nc.dram_tensor("g_rep", [128, C], F32, kind="ExternalInput").ap()
        b_d = nc.dram_tensor("b_rep", [128, C], F32, kind="ExternalInput").ap()
    if not pb0:
        pb_d = nc.dram_tensor("pb_col", [128, 1], F32, kind="ExternalInput").ap()
    if not pad0:
        pad_d = nc.dram_tensor("encoder_padding_mask", [BL, S], mybir.dt.uint8,
                               kind="ExternalInput").ap()

    with tile.TileContext(nc) as tc:
        with (
            tc.tile_pool(name="const", bufs=1) as cpool,
            tc.tile_pool(name="work", bufs=2) as wpool,
            tc.tile_pool(name="ps", bufs=2, space="PSUM") as pspool,
            tc.tile_pool(name="dram", bufs=1, space="DRAM") as dpool,
        ):
            # ---------------- constants ----------------
            ident = cpool.tile([128, 128], F32)
            make_identity(nc, ident[:])
            su32 = cpool.tile([32, 32], F32)       # su[k,m] = 1{k<m}
            make_upper_triangular(nc, su32[:], 1.0, diag=False)
            ones_row = cpool.tile([1, 128], F32)
            nc.gpsimd.memset(ones_row[:], 1.0)
            zrow = cpool.tile([1, C], F32)
            nc.gpsimd.memset(zrow[:], 0.0)
            zeros_32x128 = cpool.tile([32, 128], F32)
            nc.gpsimd.memset(zeros_32x128[:], 0.0)
            z16 = cpool.tile([128, 4], F16)
            nc.gpsimd.memset(z16[:], 0.0)
            iota_i = cpool.tile([128, 1], I32)
            nc.gpsimd.iota(iota_i[:], pattern=[[0, 1]], base=0,
                           channel_multiplier=1)
            iota_col = cpool.tile([128, 1], F32)
            nc.vector.tensor_copy(iota_col[:], iota_i[:])

            ut128 = cpool.tile([128, 128], F16)
            nc.sync.dma_start(ut128[:], ut_d[:])
            wt = cpool.tile([128, 12, C], F16)
            nc.sync.dma_start(wt[:], wt_d[:])
            pw_rep = cpool.tile([128, C], F32)
            nc.sync.dma_start(pw_rep[:], pw_d[:])
            tl_sb = cpool.tile([1, BL], I32)
            nc.sync.dma_start(tl_sb[:], tl_d[:].rearrange("(a b) -> a b", a=1))
            convb16 = g_rep = b_rep = pb_col = None
            ones16 = None
            if not cb0:
                convb16 = cpool.tile([1, C], F16)
                nc.sync.dma_start(convb16[:], convb_d[:])
                ones16 = cpool.tile([1, 128], F16)
                nc.gpsimd.memset(ones16[:], 1.0)
            if not g1b0:
                g_rep = cpool.tile([128, C], F32)
                nc.sync.dma_start(g_rep[:], g_d[:])
                b_rep = cpool.tile([128, C], F32)
                nc.sync.dma_start(b_rep[:], b_d[:])
            if not pb0:
                pb_col = cpool.tile([128, 1], F32)
                nc.sync.dma_start(pb_col[:], pb_d[:])

            R_dram = [dpool.tile([S + 1, C], F32, tag=f"Rd{b}", name=f"Rd{b}")
                      for b in range(BL)]
            csum_dram = [dpool.tile([S + 1, 1], F32, tag=f"cs{b}", name=f"cs{b}")
                         for b in range(BL)]
            offs_dram = [dpool.tile([32, C], F32, tag=f"of{b}", name=f"of{b}")
                         for b in range(BL)]

            # per-b persistent tiles
            alpha = [None] * BL
            csum_u = [None] * BL
            bend_rep = [None] * BL
            cols = [None] * BL
            idxR = [[None] * 2 for _ in range(BL)]
            idxX = [[None] * 2 for _ in range(BL)]
            blki = [[None] * 2 for _ in range(BL)]
            cprev = [[None] * 2 for _ in range(BL)]
            gr = [[None] * 2 for _ in range(BL)]
            gx = [[None] * 2 for _ in range(BL)]

            # ================= phase A+B: weight predictor =================
            def phaseAB(b):
                logits = wpool.tile([128, NBLK], F32, tag="logits",
                                    name=f"logits{b}")
                alpha[b] = wpool.tile([128, NBLK], F32, tag="alpha",
                                      name=f"alpha{b}")
                for i in range(NBLK):
                    xtw = wpool.tile([128, 4, 160], F16, tag="xtw", bufs=4,
                                     name=f"xtw{b}_{i}")
                    r0 = 128 * i - 16
                    for q in range(4):
                        cs0, cs1 = 128 * q, 128 * (q + 1)
                        if i == 0:
                            nc.sync.dma_start(xtw[:, q, 16:160],
                                              x_d[b, 0:144, cs0:cs1],
                                              transpose=True)
                        elif i == NBLK - 1:
                            nc.sync.dma_start(xtw[:, q, 0:144],
                                              x_d[b, r0:S, cs0:cs1],
                                              transpose=True)
                        else:
                            nc.sync.dma_start(xtw[:, q, 0:160],
                                              x_d[b, r0:r0 + 160, cs0:cs1],
                                              transpose=True)
                    if i == 0:
                        nc.vector.tensor_copy(
                            xtw[:, :, 15:16],
                            z16[:].rearrange("p (q o) -> p q o", o=1))
                    if i == NBLK - 1:
                        nc.vector.tensor_copy(
                            xtw[:, :, 144:145],
                            z16[:].rearrange("p (q o) -> p q o", o=1))

                    ps_h = pspool.tile([128, C], F32, tag="h", name="ps_h",
                                       bufs=3)
                    first = True
                    for k in range(3):
                        for q in range(4):
                            last = cb0 and (k == 2 and q == 3)
                            nc.tensor.matmul(ps_h[:],
                                             lhsT=xtw[:, q, 15 + k:143 + k],
                                             rhs=wt[:, k * 4 + q, :],
                                             start=first, stop=last)
                            first = False
                    if not cb0:
                        nc.tensor.matmul(ps_h[:], lhsT=ones16[:],
                                         rhs=convb16[:], start=False,
                                         stop=True)

                    # LN stats: evacuate h -> SBUF with sum(h) accumulated on
                    # the scalar engine; sum(h^2) from the SBUF copy on DVE
                    hcp = wpool.tile([128, C], F32, tag="hcp", bufs=3)
                    sh = wpool.tile([128, 1], F32, tag="sh", bufs=3)
                    nc.scalar.activation(hcp[:], ps_h[:], AF.Copy,
                                         accum_out=sh[:])
                    scr = wpool.tile([128, C], F32, tag="scr", bufs=2)
                    ssq = wpool.tile([128, 1], F32, tag="ssq", bufs=3)
                    nc.vector.scalar_tensor_tensor(scr[:], hcp[:], 1.0, hcp[:],
                                                   OP.mult, OP.mult,
                                                   accum_out=ssq[:])
                    mean = wpool.tile([128, 1], F32, tag="mean", bufs=3)
                    nc.vector.tensor_scalar_mul(mean[:], sh[:], 1.0 / C)
                    m2 = wpool.tile([128, 1], F32, tag="m2", bufs=3)
                    nc.vector.tensor_mul(m2[:], mean[:], mean[:])
                    vq = wpool.tile([128, 1], F32, tag="vq", bufs=3)
                    nc.vector.tensor_scalar(vq[:], ssq[:], 1.0 / C, LN_EPS,
                                            OP.mult, OP.add)
                    vpe = wpool.tile([128, 1], F32, tag="vpe", bufs=3)
                    nc.vector.tensor_sub(vpe[:], vq[:], m2[:])
                    stdv = wpool.tile([128, 1], F32, tag="stdv", bufs=3)
                    nc.scalar.sqrt(stdv[:], vpe[:])
                    rstd = wpool.tile([128, 1], F32, tag="rstd", bufs=3)
                    nc.vector.reciprocal(rstd[:], stdv[:])
                    negmurs = wpool.tile([128, 1], F32, tag="nmr", bufs=3)
                    nc.vector.tensor_scalar(negmurs[:], mean[:], rstd[:, 0:1],
                                            -1.0, OP.mult, OP.mult)

                    gel = wpool.tile([128, C], F32, tag="gel", bufs=3)
                    if g1b0:
                        nc.scalar.activation(gel[:], hcp[:], AF.Gelu,
                                             bias=negmurs[:, 0:1],
                                             scale=rstd[:, 0:1])
                    else:
                        z = wpool.tile([128, C], F32, tag="z", bufs=2)
                        nc.scalar.activation(z[:], hcp[:], AF.Identity,
                                             bias=negmurs[:, 0:1],
                                             scale=rstd[:, 0:1])
                        u = wpool.tile([128, C], F32, tag="u", bufs=2)
                        nc.vector.tensor_mul(u[:], z[:], g_rep[:])
                        u2 = wpool.tile([128, C], F32, tag="u2", bufs=2)
                        nc.gpsimd.tensor_add(u2[:], u[:], b_rep[:])
                        nc.scalar.activation(gel[:], u2[:], AF.Gelu)

                    scr2 = wpool.tile([128, C], F32, tag="scr2", bufs=2)
                    nc.vector.scalar_tensor_tensor(scr2[:], gel[:], 1.0,
                                                   pw_rep[:], OP.mult, OP.mult,
                                                   accum_out=logits[:, i:i + 1])

                # batched sigmoid (one Gelu->Sigmoid table swap per item)
                if pad0:
                    if pb0:
                        nc.scalar.activation(alpha[b][:], logits[:], AF.Sigmoid)
                    else:
                        nc.scalar.activation(alpha[b][:], logits[:], AF.Sigmoid,
                                             bias=pb_col[:, 0:1])
                else:
                    araw = wpool.tile([128, NBLK], F32, tag="araw")
                    if pb0:
                        nc.scalar.activation(araw[:], logits[:], AF.Sigmoid)
                    else:
                        nc.scalar.activation(araw[:], logits[:], AF.Sigmoid,
                                             bias=pb_col[:, 0:1])
                    padu8 = wpool.tile([128, NBLK], mybir.dt.uint8, tag="padu8")
                    nc.sync.dma_start(padu8[:],
                                      pad_d[b].rearrange("(i p) -> p i", p=128))
                    padf = wpool.tile([128, NBLK], F32, tag="padf")
                    nc.vector.tensor_copy(padf[:], padu8[:])
                    invpad = wpool.tile([128, NBLK], F32, tag="invpad")
                    nc.vector.tensor_scalar(invpad[:], padf[:], -1.0, 1.0,
                                            OP.mult, OP.add)
                    nc.vector.tensor_mul(alpha[b][:], araw[:], invpad[:])

            # ================= csum of alpha + per-batch scalars ===========
            def csum_scale(b):
                ps_at = pspool.tile([32, 128], F32, tag="pss", name="ps_at",
                                    bufs=2)
                nc.tensor.transpose(out=ps_at[:], in_=alpha[b][:],
                                    identity=ident[:])
                aT = wpool.tile([32, 128], F32, tag="aT")
                nc.scalar.copy(aT[:], ps_at[:])
                csum_u[b] = wpool.tile([32, 128], F32, tag="csumu",
                                       name=f"csumu{b}")
                nc.vector.tensor_tensor_scan(csum_u[b][:], zeros_32x128[:],
                                             aT[:], 0.0, OP.add, OP.add)
                btot = wpool.tile([32, 1], F32, tag="btot")
                nc.vector.tensor_copy(btot[:], csum_u[b][:, 127:128])
                ps_bo = pspool.tile([32, 1], F32, tag="pss", name="ps_bo",
                                    bufs=2)
                nc.tensor.matmul(ps_bo[:], lhsT=su32[:], rhs=btot[:],
                                 start=True, stop=True)
                boff = wpool.tile([32, 1], F32, tag="boff")
                nc.scalar.copy(boff[:], ps_bo[:])
                nc.vector.tensor_scalar_add(csum_u[b][:], csum_u[b][:],
                                            boff[:, 0:1])
                bend = wpool.tile([32, 1], F32, tag="bend")
                nc.vector.tensor_copy(bend[:], csum_u[b][:, 127:128])

                # bend replicated to all 128 partitions
                ps_bt = pspool.tile([32, 32], F32, tag="pss", name="ps_bt",
                                    bufs=2)
                nc.tensor.transpose(out=ps_bt[0:1, 0:32], in_=bend[:],
                                    identity=ident[0:32, 0:32])
                brow = wpool.tile([1, 32], F32, tag="brow")
                nc.scalar.copy(brow[:], ps_bt[0:1, 0:32])
                ps_br = pspool.tile([128, 32], F32, tag="pss", name="ps_br",
                                    bufs=2)
                nc.tensor.matmul(ps_br[:], lhsT=ones_row[:], rhs=brow[:],
                                 start=True, stop=True)
                bend_rep[b] = wpool.tile([128, 32], F32, tag="bendrep",
                                         name=f"bendrep{b}")
                nc.scalar.copy(bend_rep[b][:], ps_br[:])

                # csum -> DRAM (for csum[s-1] gathers)
                nc.sync.dma_start(csum_dram[b][0:1, :], zrow[:, 0:1])
                nc.sync.dma_start(
                    csum_dram[b][1:S + 1, :].rearrange("(p f) o -> p (f o)",
                                                       p=32),
                    csum_u[b][:])

                # per-batch scalars
                sc = wpool.tile([1, 8], F32, tag="scal")
                nc.sync.dma_start(sc[:, 0:1], csum_u[b][31:32, 127:128])
                lf = wpool.tile([1, 1], F32, tag="lf")
                nc.vector.tensor_copy(lf[:], tl_sb[:, b:b + 1])
                nc.vector.tensor_scalar_add(sc[:, 1:2], lf[:], CIF_EPS)
                nc.vector.reciprocal(sc[:, 2:3], sc[:, 0:1])
                nc.vector.tensor_mul(sc[:, 3:4], sc[:, 1:2], sc[:, 2:3])
                nc.vector.reciprocal(sc[:, 4:5], sc[:, 1:2])
                nc.vector.tensor_mul(sc[:, 5:6], sc[:, 0:1], sc[:, 4:5])
                nc.vector.tensor_scalar_mul(sc[:, 6:7], sc[:, 3:4], -1.0)
                nc.vector.tensor_copy(sc[:, 7:8], lf[:])
                ps_sc = pspool.tile([128, 8], F32, tag="pss", name="ps_sc",
                                    bufs=2)
                nc.tensor.matmul(ps_sc[:], lhsT=ones_row[:], rhs=sc[:],
                                 start=True, stop=True)
                cols[b] = wpool.tile([128, 8], F32, tag="cols",
                                     name=f"cols{b}")
                nc.scalar.copy(cols[b][:], ps_sc[:])

            # ================= R prefix scan ===============================
            def rscan(b):
                bs_sb = wpool.tile([32, C], F32, tag="bssb", name=f"bssb{b}")
                nc.sync.dma_start(R_dram[b][0:1, :], zrow[:])
                for i in range(NBLK):
                    xin2 = wpool.tile([128, C], F16, tag="xin2", bufs=3,
                                      name=f"xin2_{b}_{i}")
                    nc.sync.dma_start(xin2[:], x_d[b, 128 * i:128 * (i + 1), :])
                    uta = wpool.tile([128, 128], F16, tag="uta", bufs=3)
                    nc.vector.tensor_scalar_mul(uta[:], ut128[:],
                                                alpha[b][:, i:i + 1])
                    ps_rp = pspool.tile([128, C], F32, tag="rp", name="ps_rp",
                                        bufs=2)
                    nc.tensor.matmul(ps_rp[:], lhsT=uta[:], rhs=xin2[:],
                                     start=True, stop=True)
                    rp_sb = wpool.tile([128, C], F32, tag="rpsb", bufs=3)
                    nc.scalar.copy(rp_sb[:], ps_rp[:])
                    nc.sync.dma_start(
                        R_dram[b][1 + 128 * i:1 + 128 * (i + 1), :], rp_sb[:])
                    nc.sync.dma_start(bs_sb[i:i + 1, :], rp_sb[127:128, :])
                ps_off = pspool.tile([32, C], F32, tag="pso", name="ps_off",
                                     bufs=1)
                nc.tensor.matmul(ps_off[:], lhsT=su32[:], rhs=bs_sb[:],
                                 start=True, stop=True)
                offs_sb = wpool.tile([32, C], F32, tag="offsb")
                nc.scalar.copy(offs_sb[:], ps_off[:])
                nc.sync.dma_start(offs_dram[b][:], offs_sb[:])

            # ================= searchsorted + gathers ======================
            def search(b):
                invscale_c = cols[b][:, 5:6]
                for kk in range(2):
                    idxR[b][kk] = wpool.tile([128, NT], I32, tag=f"idxR{kk}",
                                             name=f"idxR{b}{kk}")
                    idxX[b][kk] = wpool.tile([128, NT], I32, tag=f"idxX{kk}",
                                             name=f"idxX{b}{kk}")
                    blki[b][kk] = wpool.tile([128, NT], I32, tag=f"blki{kk}",
                                             name=f"blki{b}{kk}")
                    cprev[b][kk] = wpool.tile([128, NT], F32, tag=f"cprev{kk}",
                                              name=f"cprev{b}{kk}")
                    gr[b][kk] = wpool.tile([128, NT, C], F32, tag=f"gr{kk}",
                                           name=f"gr{b}{kk}", bufs=2)
                    gx[b][kk] = wpool.tile([128, NT, C], F16, tag=f"gx{kk}",
                                           name=f"gx{b}{kk}", bufs=2)
                x_flat = x_d.rearrange("b s c -> (b s) c")
                for kk, cmp_op in ((0, OP.is_le), (1, OP.is_lt)):
                    for j in range(NT):
                        tau = wpool.tile([128, 1], F32, tag="tau", bufs=2)
                        shift = float(128 * j + kk)
                        nc.vector.tensor_scalar(tau[:], iota_col[:], shift,
                                                invscale_c, OP.add, OP.mult)
                        cmp1 = wpool.tile([128, 32], F32, tag="cmp1", bufs=2)
                        bcnt = wpool.tile([128, 1], F32, tag="bcnt", bufs=2)
                        nc.vector.tensor_scalar(cmp1[:], bend_rep[b][:],
                                                tau[:, 0:1], 0.0, cmp_op,
                                                OP.add, accum_out=bcnt[:])
                        oh1 = wpool.tile([128, 32], F32, tag="oh1", bufs=2)
                        nc.vector.tensor_scalar(oh1[:, 0:1], cmp1[:, 0:1],
                                                -1.0, 1.0, OP.mult, OP.add)
                        nc.vector.tensor_sub(oh1[:, 1:32], cmp1[:, 0:31],
                                             cmp1[:, 1:32])
                        ps_t = pspool.tile([32, 128], F32, tag="pss",
                                           name="ps_t", bufs=2)
                        nc.tensor.transpose(out=ps_t[:], in_=oh1[:],
                                            identity=ident[:])
                        oh1T = wpool.tile([32, 128], F32, tag="oh1T", bufs=2)
                        nc.scalar.copy(oh1T[:], ps_t[:])
                        ps_sel = pspool.tile([128, 128], F32, tag="pss",
                                             name="ps_sel", bufs=2)
                        nc.tensor.matmul(ps_sel[:], lhsT=oh1T[:],
                                         rhs=csum_u[b][:], start=True,
                                         stop=True)
                        cmp2 = wpool.tile([128, 128], F32, tag="cmp2", bufs=2)
                        cnt = wpool.tile([128, 1], F32, tag="cnt", bufs=2)
                        nc.vector.tensor_scalar(cmp2[:], ps_sel[:],
                                                tau[:, 0:1], 0.0, cmp_op,
                                                OP.add, accum_out=cnt[:])
                        sidx = wpool.tile([128, 1], F32, tag="sidx", bufs=2)
                        nc.vector.tensor_scalar(sidx[:], bcnt[:], 128.0,
                                                cnt[:, 0:1], OP.mult, OP.add)
                        idr_f = wpool.tile([128, 1], F32, tag="idrf", bufs=2)
                        nc.vector.tensor_scalar_min(idr_f[:], sidx[:],
                                                    float(S))
                        nc.vector.tensor_copy(idxR[b][kk][:, j:j + 1],
                                              idr_f[:])
                        idx_f = wpool.tile([128, 1], F32, tag="idxf", bufs=2)
                        nc.vector.tensor_scalar_min(idx_f[:], sidx[:],
                                                    float(S - 1))
                        nc.vector.tensor_copy(idxX[b][kk][:, j:j + 1],
                                              idx_f[:])
                        zc = wpool.tile([128, 1], F32, tag="zc", bufs=2)
                        nc.vector.tensor_scalar(zc[:], cnt[:], 0.0, None,
                                                OP.is_equal)
                        blkf = wpool.tile([128, 1], F32, tag="blkf", bufs=2)
                        nc.vector.tensor_sub(blkf[:], bcnt[:], zc[:])
                        nc.vector.tensor_copy(blki[b][kk][:, j:j + 1],
                                              blkf[:])

                        # gathers
                        nc.gpsimd.indirect_dma_start(
                            out=cprev[b][kk][:, j:j + 1], out_offset=None,
                            in_=csum_dram[b][:],
                            in_offset=bass.IndirectOffsetOnAxis(
                                ap=idxR[b][kk][:, j:j + 1], axis=0))
                        nc.gpsimd.indirect_dma_start(
                            out=gr[b][kk][:, j, :], out_offset=None,
                            in_=R_dram[b][:],
                            in_offset=bass.IndirectOffsetOnAxis(
                                ap=idxR[b][kk][:, j:j + 1], axis=0))
                        nc.gpsimd.indirect_dma_start(
                            out=gr[b][kk][:, j, :], out_offset=None,
                            in_=offs_dram[b][:],
                            in_offset=bass.IndirectOffsetOnAxis(
                                ap=blki[b][kk][:, j:j + 1], axis=0),
                            bounds_check=31, oob_is_err=False,
                            compute_op=OP.add)
                        nc.gpsimd.indirect_dma_start(
                            out=gx[b][kk][:, j, :], out_offset=None,
                            in_=x_flat,
                            in_offset=bass.IndirectOffsetOnAxis(
                                ap=idxX[b][kk][:, j:j + 1], axis=0),
                            element_offset=b * S * C)

            # ================= combine & write out =========================
            def combine(b):
                scale_c = cols[b][:, 3:4]
                negscale_c = cols[b][:, 6:7]
                L_c = cols[b][:, 7:8]
                for j in range(NT):
                    tcol = wpool.tile([128, 1], F32, tag="tcol", bufs=2)
                    nc.vector.tensor_scalar_add(tcol[:], iota_col[:],
                                                float(128 * j))
                    valid = wpool.tile([128, 1], F32, tag="valid", bufs=2)
                    nc.vector.tensor_scalar(valid[:], tcol[:], L_c, None,
                                            OP.is_lt)
                    c1 = wpool.tile([128, 1], F32, tag="c1", bufs=2)
                    nc.vector.tensor_scalar(c1[:], cprev[b][0][:, j:j + 1],
                                            scale_c, tcol[:, 0:1], OP.mult,
                                            OP.subtract)
                    nc.vector.tensor_mul(c1[:], c1[:], valid[:])
                    t1col = wpool.tile([128, 1], F32, tag="t1col", bufs=2)
                    nc.vector.tensor_scalar_add(t1col[:], tcol[:], 1.0)
                    c2 = wpool.tile([128, 1], F32, tag="c2", bufs=2)
                    nc.vector.tensor_scalar(c2[:], cprev[b][1][:, j:j + 1],
                                            negscale_c, t1col[:, 0:1],
                                            OP.mult, OP.add)
                    nc.vector.tensor_mul(c2[:], c2[:], valid[:])
                    sv = wpool.tile([128, 1], F32, tag="sv", bufs=2)
                    nc.vector.tensor_mul(sv[:], scale_c, valid[:])

                    d = wpool.tile([128, C], F32, tag="d", bufs=2)
                    nc.vector.tensor_sub(d[:], gr[b][1][:, j, :],
                                         gr[b][0][:, j, :])
                    t0 = wpool.tile([128, C], F32, tag="t0", bufs=2)
                    nc.scalar.activation(t0[:], gx[b][0][:, j, :], AF.Copy,
                                         scale=c1[:, 0:1])
                    t1 = wpool.tile([128, C], F32, tag="t1", bufs=2)
                    nc.vector.scalar_tensor_tensor(t1[:], gx[b][1][:, j, :],
                                                   c2[:, 0:1], t0[:], OP.mult,
                                                   OP.add)
                    ot = wpool.tile([128, C], F32, tag="ot", bufs=2)
                    nc.vector.scalar_tensor_tensor(ot[:], d[:], sv[:, 0:1],
                                                   t1[:], OP.mult, OP.add)
                    nc.sync.dma_start(out_d[b, 128 * j:128 * (j + 1), :],
                                      ot[:])

            # ================= emission order ==============================
            KSTAGE = int(os.environ.get("KSTAGE", "9"))
            phaseAB(0)
            if KSTAGE >= 1:
                csum_scale(0)
            if KSTAGE >= 2:
                rscan(0)
            if KSTAGE >= 3:
                phaseAB(1)
            if KSTAGE >= 4:
                search(0)
            if KSTAGE >= 5:
                combine(0)
            if KSTAGE >= 6:
                csum_scale(1)
                rscan(1)
                search(1)
                combine(1)

    nc.compile()
    return nc


_prog_cache = {}


def _get_prog(flags=(True, True, True, True)):
    if flags not in _prog_cache:
        _prog_cache[flags] = build_program(*flags)
    return _prog_cache[flags]


def make_in_maps(inputs):
    """Host-side preprocessing: shard, cast x to fp16, pre-transpose conv_w."""
    x = np.asarray(inputs["x"], np.float32)
    pad = np.asarray(inputs["encoder_padding_mask"]).astype(np.uint8)
    tl = np.asarray(inputs["target_lengths"]).astype(np.int32)
    conv_w = np.asarray(inputs["conv_w"], np.float32)
    conv_b = np.asarray(inputs["conv_b"], np.float32)
    ln_g = np.asarray(inputs["ln_g"], np.float32)
    ln_b = np.asarray(inputs["ln_b"], np.float32)
    proj_w = np.asarray(inputs["proj_w"], np.float32)
    proj_b = np.asarray(inputs["proj_b"], np.float32)

    g1b0 = bool(np.all(ln_g == 1.0) and np.all(ln_b == 0.0))
    cb0 = bool(np.all(conv_b == 0.0))
    pb0 = bool(np.all(proj_b == 0.0))
    pad0 = bool(not pad.any())
    flags = (g1b0, cb0, pb0, pad0)

    x16 = x.astype(np.float16)
    w16 = conv_w.astype(np.float16)
    wt = np.empty((128, 12, C), np.float16)
    for k in range(3):
        for q in range(4):
            wt[:, k * 4 + q, :] = w16[:, 128 * q:128 * (q + 1), k].T
    pw_rep = np.ascontiguousarray(
        np.broadcast_to(proj_w[:, 0][None, :], (128, C)).astype(np.float32))
    ut128 = np.triu(np.ones((128, 128), np.float16), 0)

    in_maps = []
    for core in range(NCORES):
        lo, hi = core * BL, (core + 1) * BL
        m = {
            "x": np.ascontiguousarray(x16[lo:hi]),
            "wt": wt,
            "pw_rep": pw_rep,
            "ut128": ut128,
            "target_lengths": np.ascontiguousarray(tl[lo:hi]),
        }
        if not cb0:
            m["convb16"] = conv_b.astype(np.float16)[None, :]
        if not g1b0:
            m["g_rep"] = np.ascontiguousarray(
                np.broadcast_to(ln_g[None, :], (128, C)).astype(np.float32))
            m["b_rep"] = np.ascontiguousarray(
                np.broadcast_to(ln_b[None, :], (128, C)).astype(np.float32))
        if not pb0:
            m["pb_col"] = np.full((128, 1), float(proj_b[0]), np.float32)
        if not pad0:
            m["encoder_padding_mask"] = np.ascontiguousarray(pad[lo:hi])
        in_maps.append(m)
    return flags, in_maps


def kernel(**inputs):
    flags, in_maps = make_in_maps(inputs)
    nc = _get_prog(flags)
    res = run_bass_kernel_spmd(nc, in_maps, core_ids=list(range(NCORES)))
    out = np.concatenate([res.results[c]["out"] for c in range(NCORES)],
                         axis=0)
    return out.astype(np.float32)


if __name__ == "__main__":
    import reference as ref
    import jax
    jax.config.update("jax_platforms", "cpu")
    inputs = ref.setup_inputs()
    actual = kernel(**{k: np.asarray(v) for k, v in inputs.items()})
    print("kernel output", actual.shape, actual.dtype)


# revision 9
# speedup vs baseline: 1.5561x; 1.4668x over previous
"""Trainium2 Bass kernel for a CIF (continuous-integrate-and-fire) layer.

Takes FULL inputs (B=16), shards batch-parallel across 8 NeuronCores
(2 batch items per core), runs one Bass/Tile program per core via
run_bass_kernel_spmd, and gathers the full (16, 512, 512) output.

Math: the CIF scatter is reformulated as interval overlap,
  A[s,t] = clamp(csum[s]-t,0,1) - clamp(csum[s-1]-t,0,1)
which telescopes into
  out[t] = scale*(Ru[s2-1]-Ru[s1-1]) + (1+t-c[s2-1])*x[s2] + (c[s1-1]-t)*x[s1]
with Ru = prefix-sum of alpha_u * x (unscaled), c = scale*csum_u,
s1 = first s with scale*csum_u[s] > t, s2 = first s with scale*csum_u[s] >= t+1.
Exact when every step fires at most once (alpha <= 1 after scaling).

Perf notes vs the fp32r baseline:
- x fed as fp16 from the host: conv/scan matmuls run fp16, x windows are
  loaded pre-transposed via the DMA XBAR (no PE transposes / PSUM evac).
- conv weights pre-transposed+cast on the host -> no setup transposes.
- Predictor: conv PSUM -> fused Gelu(scale=rstd, bias=-mu*rstd) when
  ln_g==1, ln_b==0 (true for the reference inputs); sigmoid batched per
  batch item so the scalar engine keeps the Gelu table loaded.
- searchsorted: csum[s-1] and R block offsets fetched by indirect DMA
  gathers (offsets accumulated into the R gather with OOB-skip for the
  "before block 0" case) instead of PE select matmuls.
"""

import os
import numpy as np

try:
    import concourse.bass as bass
except ImportError:
    import sys
    sys.path.insert(0, "/opt/trn_rl_repo")
    import concourse.bass as bass

import concourse.tile as tile
from concourse import bacc, mybir
from concourse.bass_utils import run_bass_kernel_spmd
from concourse.masks import make_identity, make_upper_triangular

F32 = mybir.dt.float32
F16 = mybir.dt.float16
I32 = mybir.dt.int32
AF = mybir.ActivationFunctionType
OP = mybir.AluOpType

B, S, C, T = 16, 4096, 512, 512
NCORES = 8
BL = B // NCORES          # batch items per core
NBLK = S // 128           # 32 s-blocks per batch item
NT = T // 128             # 4 t-tiles
CIF_EPS = 1e-4
LN_EPS = 1e-5


def build_program(g1b0=True, cb0=True, pb0=True, pad0=True):
    nc = bacc.Bacc("TRN2", target_bir_lowering=False, debug=False)

    x_d = nc.dram_tensor("x", [BL, S, C], F16, kind="ExternalInput").ap()
    wt_d = nc.dram_tensor("wt", [128, 12, C], F16, kind="ExternalInput").ap()
    pw_d = nc.dram_tensor("pw_rep", [128, C], F32, kind="ExternalInput").ap()
    ut_d = nc.dram_tensor("ut128", [128, 128], F16, kind="ExternalInput").ap()
    tl_d = nc.dram_tensor("target_lengths", [BL], I32, kind="ExternalInput").ap()
    out_d = nc.dram_tensor("out", [BL, T, C], F32, kind="ExternalOutput").ap()
    convb_d = g_d = b_d = pb_d = pad_d = None
    if not cb0:
        convb_d = nc.dram_tensor("convb16", [1, C], F16, kind="ExternalInput").ap()
    if not g1b0:
        g_d = nc.dram_tensor("g_rep", [128, C], F32, kind="ExternalInput").ap()
        b_d = nc.dram_tensor("b_rep", [128, C], F32, kind="ExternalInput").ap()
    if not pb0:
        pb_d = nc.dram_tensor("pb_col", [128, 1], F32, kind="ExternalInput").ap()
    if not pad0:
        pad_d = nc.dram_tensor("encoder_padding_mask", [BL, S], mybir.dt.uint8,
                               kind="ExternalInput").ap()

    with tile.TileContext(nc) as tc:
        with (
            tc.tile_pool(name="const", bufs=1) as cpool,
            tc.tile_pool(name="work", bufs=2) as wpool,
            tc.tile_pool(name="ps", bufs=2, space="PSUM") as pspool,
            tc.tile_pool(name="dram", bufs=1, space="DRAM") as dpool,
        ):
            # ---------------- constants ----------------
            ident = cpool.tile([128, 128], F32)
            make_identity(nc, ident[:])
            su32 = cpool.tile([32, 32], F32)       # su[k,m] = 1{k<m}
            make_upper_triangular(nc, su32[:], 1.0, diag=False)
            ones_row = cpool.tile([1, 128], F32)
            nc.gpsimd.memset(ones_row[:], 1.0)
            zrow = cpool.tile([1, C], F32)
            nc.gpsimd.memset(zrow[:], 0.0)
            zeros_32x128 = cpool.tile([32, 128], F32)
            nc.gpsimd.memset(zeros_32x128[:], 0.0)
            z16 = cpool.tile([128, 4], F16)
            nc.gpsimd.memset(z16[:], 0.0)
            iota_i = cpool.tile([128, 1], I32)
            nc.gpsimd.iota(iota_i[:], pattern=[[0, 1]], base=0,
                           channel_multiplier=1)
            iota_col = cpool.tile([128, 1], F32)
            nc.vector.tensor_copy(iota_col[:], iota_i[:])

            ut128 = cpool.tile([128, 128], F16)
            nc.sync.dma_start(ut128[:], ut_d[:])
            wt = cpool.tile([128, 12, C], F16)
            nc.sync.dma_start(wt[:], wt_d[:])
            pw_rep = cpool.tile([128, C], F32)
            nc.sync.dma_start(pw_rep[:], pw_d[:])
            tl_sb = cpool.tile([1, BL], I32)
            nc.sync.dma_start(tl_sb[:], tl_d[:].rearrange("(a b) -> a b", a=1))
            convb16 = g_rep = b_rep = pb_col = None
            ones16 = None
            if not cb0:
                convb16 = cpool.tile([1, C], F16)
                nc.sync.dma_start(convb16[:], convb_d[:])
                ones16 = cpool.tile([1, 128], F16)
                nc.gpsimd.memset(ones16[:], 1.0)
            if not g1b0:
                g_rep = cpool.tile([128, C], F32)
                nc.sync.dma_start(g_rep[:], g_d[:])
                b_rep = cpool.tile([128, C], F32)
                nc.sync.dma_start(b_rep[:], b_d[:])
            if not pb0:
                pb_col = cpool.tile([128, 1], F32)
                nc.sync.dma_start(pb_col[:], pb_d[:])

            R_dram = [dpool.tile([S + 1, C], F32, tag=f"Rd{b}", name=f"Rd{b}")
                      for b in range(BL)]
            csum_dram = [dpool.tile([S + 1, 1], F32, tag=f"cs{b}", name=f"cs{b}")
                         for b in range(BL)]
            offs_dram = [dpool.tile([32, C], F32, tag=f"of{b}", name=f"of{b}")
                         for b in range(BL)]

            # per-b persistent tiles
            alpha = [None] * BL
            csum_u = [None] * BL
            bend_rep = [None] * BL
            cols = [None] * BL
            idxR = [[None] * 2 for _ in range(BL)]
            idxX = [[None] * 2 for _ in range(BL)]
            blki = [[None] * 2 for _ in range(BL)]
            cprev = [[None] * 2 for _ in range(BL)]
            gr = [[None] * 2 for _ in range(BL)]
            gx = [[None] * 2 for _ in range(BL)]

            # ================= phase A+B: weight predictor =================
            def phaseAB(b):
                logits = wpool.tile([128, NBLK], F32, tag="logits",
                                    name=f"logits{b}")
                alpha[b] = wpool.tile([128, NBLK], F32, tag="alpha",
                                      name=f"alpha{b}")
                NSB = 8            # conv blocks per transposed superblock
                NG = NBLK // NSB   # 4 superblocks
                W = NSB * 128 + 32  # 1056 window columns
                for g in range(NG):
                  xtw = wpool.tile([128, 4, W], F16, tag="xtw", bufs=2,
                                   name=f"xtw{b}_{g}")
                  r0 = NSB * 128 * g - 16
                  for q in range(4):
                      cs0, cs1 = 128 * q, 128 * (q + 1)
                      eng = nc.scalar if q % 2 else nc.sync
                      if g == 0:
                          eng.dma_start(xtw[:, q, 16:W],
                                        x_d[b, 0:W - 16, cs0:cs1],
                                        transpose=True)
                      elif g == NG - 1:
                          eng.dma_start(xtw[:, q, 0:W - 16],
                                        x_d[b, r0:S, cs0:cs1],
                                        transpose=True)
                      else:
                          eng.dma_start(xtw[:, q, 0:W],
                                        x_d[b, r0:r0 + W, cs0:cs1],
                                        transpose=True)
                  if g == 0:
                      nc.vector.tensor_copy(
                          xtw[:, :, 15:16],
                          z16[:].rearrange("p (q o) -> p q o", o=1))
                  if g == NG - 1:
                      nc.vector.tensor_copy(
                          xtw[:, :, W - 16:W - 15],
                          z16[:].rearrange("p (q o) -> p q o", o=1))

                  for m in range(NSB):
                    i = NSB * g + m
                    c0 = 128 * m + 15
                    ps_h = pspool.tile([128, C], F32, tag="h", name="ps_h",
                                       bufs=3)
                    first = True
                    for k in range(3):
                        for q in range(4):
                            last = cb0 and (k == 2 and q == 3)
                            nc.tensor.matmul(ps_h[:],
                                             lhsT=xtw[:, q, c0 + k:c0 + 128 + k],
                                             rhs=wt[:, k * 4 + q, :],
                                             start=first, stop=last)
                            first = False
                    if not cb0:
                        nc.tensor.matmul(ps_h[:], lhsT=ones16[:],
                                         rhs=convb16[:], start=False,
                                         stop=True)

                    # LN stats: evacuate h -> SBUF with sum(h) accumulated on
                    # the scalar engine; sum(h^2) from the SBUF copy on DVE
                    hcp = wpool.tile([128, C], F32, tag="hcp", bufs=3)
                    sh = wpool.tile([128, 1], F32, tag="sh", bufs=3)
                    nc.scalar.activation(hcp[:], ps_h[:], AF.Copy,
                                         accum_out=sh[:])
                    scr = wpool.tile([128, C], F32, tag="scr", bufs=2)
                    ssq = wpool.tile([128, 1], F32, tag="ssq", bufs=3)
                    nc.vector.scalar_tensor_tensor(scr[:], hcp[:], 1.0, hcp[:],
                                                   OP.mult, OP.mult,
                                                   accum_out=ssq[:])
                    mean = wpool.tile([128, 1], F32, tag="mean", bufs=3)
                    nc.vector.tensor_scalar_mul(mean[:], sh[:], 1.0 / C)
                    m2 = wpool.tile([128, 1], F32, tag="m2", bufs=3)
                    nc.vector.tensor_mul(m2[:], mean[:], mean[:])
                    vq = wpool.tile([128, 1], F32, tag="vq", bufs=3)
                    nc.vector.tensor_scalar(vq[:], ssq[:], 1.0 / C, LN_EPS,
                                            OP.mult, OP.add)
                    vpe = wpool.tile([128, 1], F32, tag="vpe", bufs=3)
                    nc.vector.tensor_sub(vpe[:], vq[:], m2[:])
                    # rsqrt via int bithack + 2 Newton steps, all on DVE
                    # (keeps the scalar engine's Gelu table loaded)
                    hsh = wpool.tile([128, 1], I32, tag="hsh", bufs=3)
                    nc.vector.tensor_scalar(hsh[:], vpe[:].bitcast(I32), 1,
                                            None, OP.logical_shift_right)
                    y0i = wpool.tile([128, 1], I32, tag="y0i", bufs=3)
                    nc.vector.tensor_scalar(y0i[:], hsh[:], -1, 0x5f3759df,
                                            OP.mult, OP.add)
                    rstd = wpool.tile([128, 1], F32, tag="rstd", bufs=3)
                    nc.vector.tensor_copy(rstd[:], y0i[:].bitcast(F32))
                    nt = wpool.tile([128, 1], F32, tag="nt", bufs=3)
                    for _ in range(2):
                        nc.vector.tensor_mul(nt[:], rstd[:], rstd[:])
                        nc.vector.tensor_mul(nt[:], nt[:], vpe[:])
                        nc.vector.tensor_scalar(nt[:], nt[:], -0.5, 1.5,
                                                OP.mult, OP.add)
                        nc.vector.tensor_mul(rstd[:], rstd[:], nt[:])
                    negmurs = wpool.tile([128, 1], F32, tag="nmr", bufs=3)
                    nc.vector.tensor_scalar(negmurs[:], mean[:], rstd[:, 0:1],
                                            -1.0, OP.mult, OP.mult)

                    gel = wpool.tile([128, C], F32, tag="gel", bufs=3)
                    if g1b0:
                        nc.scalar.activation(gel[:], hcp[:], AF.Gelu,
                                             bias=negmurs[:, 0:1],
                                             scale=rstd[:, 0:1])
                    else:
                        z = wpool.tile([128, C], F32, tag="z", bufs=2)
                        nc.scalar.activation(z[:], hcp[:], AF.Identity,
                                             bias=negmurs[:, 0:1],
                                             scale=rstd[:, 0:1])
                        u = wpool.tile([128, C], F32, tag="u", bufs=2)
                        nc.vector.tensor_mul(u[:], z[:], g_rep[:])
                        u2 = wpool.tile([128, C], F32, tag="u2", bufs=2)
                        nc.gpsimd.tensor_add(u2[:], u[:], b_rep[:])
                        nc.scalar.activation(gel[:], u2[:], AF.Gelu)

                    scr2 = wpool.tile([128, C], F32, tag="scr2", bufs=2)
                    nc.vector.scalar_tensor_tensor(scr2[:], gel[:], 1.0,
                                                   pw_rep[:], OP.mult, OP.mult,
                                                   accum_out=logits[:, i:i + 1])

                # batched sigmoid (one Gelu->Sigmoid table swap per item)
                if pad0:
                    if pb0:
                        nc.scalar.activation(alpha[b][:], logits[:], AF.Sigmoid)
                    else:
                        nc.scalar.activation(alpha[b][:], logits[:], AF.Sigmoid,
                                             bias=pb_col[:, 0:1])
                else:
                    araw = wpool.tile([128, NBLK], F32, tag="araw")
                    if pb0:
                        nc.scalar.activation(araw[:], logits[:], AF.Sigmoid)
                    else:
                        nc.scalar.activation(araw[:], logits[:], AF.Sigmoid,
                                             bias=pb_col[:, 0:1])
                    padu8 = wpool.tile([128, NBLK], mybir.dt.uint8, tag="padu8")
                    nc.sync.dma_start(padu8[:],
                                      pad_d[b].rearrange("(i p) -> p i", p=128))
                    padf = wpool.tile([128, NBLK], F32, tag="padf")
                    nc.vector.tensor_copy(padf[:], padu8[:])
                    invpad = wpool.tile([128, NBLK], F32, tag="invpad")
                    nc.vector.tensor_scalar(invpad[:], padf[:], -1.0, 1.0,
                                            OP.mult, OP.add)
                    nc.vector.tensor_mul(alpha[b][:], araw[:], invpad[:])

            # ================= csum of alpha + per-batch scalars ===========
            def csum_scale(b):
                ps_at = pspool.tile([32, 128], F32, tag="pss", name="ps_at",
                                    bufs=2)
                nc.tensor.transpose(out=ps_at[:], in_=alpha[b][:],
                                    identity=ident[:])
                aT = wpool.tile([32, 128], F32, tag="aT")
                nc.scalar.copy(aT[:], ps_at[:])
                csum_u[b] = wpool.tile([32, 128], F32, tag="csumu",
                                       name=f"csumu{b}")
                nc.vector.tensor_tensor_scan(csum_u[b][:], zeros_32x128[:],
                                             aT[:], 0.0, OP.add, OP.add)
                btot = wpool.tile([32, 1], F32, tag="btot")
                nc.vector.tensor_copy(btot[:], csum_u[b][:, 127:128])
                ps_bo = pspool.tile([32, 1], F32, tag="pss", name="ps_bo",
                                    bufs=2)
                nc.tensor.matmul(ps_bo[:], lhsT=su32[:], rhs=btot[:],
                                 start=True, stop=True)
                boff = wpool.tile([32, 1], F32, tag="boff")
                nc.scalar.copy(boff[:], ps_bo[:])
                nc.vector.tensor_scalar_add(csum_u[b][:], csum_u[b][:],
                                            boff[:, 0:1])
                bend = wpool.tile([32, 1], F32, tag="bend")
                nc.vector.tensor_copy(bend[:], csum_u[b][:, 127:128])

                # bend replicated to all 128 partitions
                ps_bt = pspool.tile([32, 32], F32, tag="pss", name="ps_bt",
                                    bufs=2)
                nc.tensor.transpose(out=ps_bt[0:1, 0:32], in_=bend[:],
                                    identity=ident[0:32, 0:32])
                brow = wpool.tile([1, 32], F32, tag="brow")
                nc.scalar.copy(brow[:], ps_bt[0:1, 0:32])
                ps_br = pspool.tile([128, 32], F32, tag="pss", name="ps_br",
                                    bufs=2)
                nc.tensor.matmul(ps_br[:], lhsT=ones_row[:], rhs=brow[:],
                                 start=True, stop=True)
                bend_rep[b] = wpool.tile([128, 32], F32, tag="bendrep",
                                         name=f"bendrep{b}")
                nc.scalar.copy(bend_rep[b][:], ps_br[:])

                # csum -> DRAM (for csum[s-1] gathers)
                nc.sync.dma_start(csum_dram[b][0:1, :], zrow[:, 0:1])
                nc.sync.dma_start(
                    csum_dram[b][1:S + 1, :].rearrange("(p f) o -> p (f o)",
                                                       p=32),
                    csum_u[b][:])

                # per-batch scalars
                sc = wpool.tile([1, 8], F32, tag="scal")
                nc.sync.dma_start(sc[:, 0:1], csum_u[b][31:32, 127:128])
                lf = wpool.tile([1, 1], F32, tag="lf")
                nc.vector.tensor_copy(lf[:], tl_sb[:, b:b + 1])
                nc.vector.tensor_scalar_add(sc[:, 1:2], lf[:], CIF_EPS)
                nc.vector.reciprocal(sc[:, 2:3], sc[:, 0:1])
                nc.vector.tensor_mul(sc[:, 3:4], sc[:, 1:2], sc[:, 2:3])
                nc.vector.reciprocal(sc[:, 4:5], sc[:, 1:2])
                nc.vector.tensor_mul(sc[:, 5:6], sc[:, 0:1], sc[:, 4:5])
                nc.vector.tensor_scalar_mul(sc[:, 6:7], sc[:, 3:4], -1.0)
                nc.vector.tensor_copy(sc[:, 7:8], lf[:])
                ps_sc = pspool.tile([128, 8], F32, tag="pss", name="ps_sc",
                                    bufs=2)
                nc.tensor.matmul(ps_sc[:], lhsT=ones_row[:], rhs=sc[:],
                                 start=True, stop=True)
                cols[b] = wpool.tile([128, 8], F32, tag="cols",
                                     name=f"cols{b}")
                nc.scalar.copy(cols[b][:], ps_sc[:])

            # ================= R prefix scan ===============================
            def rscan(b):
                bs_sb = wpool.tile([32, C], F32, tag="bssb", name=f"bssb{b}")
                nc.sync.dma_start(R_dram[b][0:1, :], zrow[:])
                for i in range(NBLK):
                    xin2 = wpool.tile([128, C], F16, tag="xin2", bufs=3,
                                      name=f"xin2_{b}_{i}")
                    nc.sync.dma_start(xin2[:], x_d[b, 128 * i:128 * (i + 1), :])
                    uta = wpool.tile([128, 128], F16, tag="uta", bufs=3)
                    nc.vector.tensor_scalar_mul(uta[:], ut128[:],
                                                alpha[b][:, i:i + 1])
                    ps_rp = pspool.tile([128, C], F32, tag="rp", name="ps_rp",
                                        bufs=2)
                    nc.tensor.matmul(ps_rp[:], lhsT=uta[:], rhs=xin2[:],
                                     start=True, stop=True)
                    rp_sb = wpool.tile([128, C], F32, tag="rpsb", bufs=3)
                    nc.scalar.copy(rp_sb[:], ps_rp[:])
                    nc.sync.dma_start(
                        R_dram[b][1 + 128 * i:1 + 128 * (i + 1), :], rp_sb[:])
                    nc.sync.dma_start(bs_sb[i:i + 1, :], rp_sb[127:128, :])
                ps_off = pspool.tile([32, C], F32, tag="pso", name="ps_off",
                                     bufs=1)
                nc.tensor.matmul(ps_off[:], lhsT=su32[:], rhs=bs_sb[:],
                                 start=True, stop=True)
                offs_sb = wpool.tile([32, C], F32, tag="offsb")
                nc.scalar.copy(offs_sb[:], ps_off[:])
                nc.sync.dma_start(offs_dram[b][:], offs_sb[:])

            # ================= searchsorted + gathers ======================
            def search(b):
                invscale_c = cols[b][:, 5:6]
                for kk in range(2):
                    idxR[b][kk] = wpool.tile([128, NT], I32, tag=f"idxR{kk}",
                                             name=f"idxR{b}{kk}")
                    idxX[b][kk] = wpool.tile([128, NT], I32, tag=f"idxX{kk}",
                                             name=f"idxX{b}{kk}")
                    blki[b][kk] = wpool.tile([128, NT], I32, tag=f"blki{kk}",
                                             name=f"blki{b}{kk}")
                    cprev[b][kk] = wpool.tile([128, NT], F32, tag=f"cprev{kk}",
                                              name=f"cprev{b}{kk}")
                    gr[b][kk] = wpool.tile([128, NT, C], F32, tag=f"gr{kk}",
                                           name=f"gr{b}{kk}", bufs=2)
                    gx[b][kk] = wpool.tile([128, NT, C], F16, tag=f"gx{kk}",
                                           name=f"gx{b}{kk}", bufs=2)
                x_flat = x_d.rearrange("b s c -> (b s) c")
                for kk, cmp_op in ((0, OP.is_le), (1, OP.is_lt)):
                    for j in range(NT):
                        tau = wpool.tile([128, 1], F32, tag="tau", bufs=2)
                        shift = float(128 * j + kk)
                        nc.vector.tensor_scalar(tau[:], iota_col[:], shift,
                                                invscale_c, OP.add, OP.mult)
                        cmp1 = wpool.tile([128, 32], F32, tag="cmp1", bufs=2)
                        bcnt = wpool.tile([128, 1], F32, tag="bcnt", bufs=2)
                        nc.vector.tensor_scalar(cmp1[:], bend_rep[b][:],
                                                tau[:, 0:1], 0.0, cmp_op,
                                                OP.add, accum_out=bcnt[:])
                        oh1 = wpool.tile([128, 32], F32, tag="oh1", bufs=2)
                        nc.vector.tensor_scalar(oh1[:, 0:1], cmp1[:, 0:1],
                                                -1.0, 1.0, OP.mult, OP.add)
                        nc.vector.tensor_sub(oh1[:, 1:32], cmp1[:, 0:31],
                                             cmp1[:, 1:32])
                        ps_t = pspool.tile([32, 128], F32, tag="pss",
                                           name="ps_t", bufs=2)
                        nc.tensor.transpose(out=ps_t[:], in_=oh1[:],
                                            identity=ident[:])
                        oh1T = wpool.tile([32, 128], F32, tag="oh1T", bufs=2)
                        nc.scalar.copy(oh1T[:], ps_t[:])
                        ps_sel = pspool.tile([128, 128], F32, tag="pss",
                                             name="ps_sel", bufs=2)
                        nc.tensor.matmul(ps_sel[:], lhsT=oh1T[:],
                                         rhs=csum_u[b][:], start=True,
                                         stop=True)
                        cmp2 = wpool.tile([128, 128], F32, tag="cmp2", bufs=2)
                        cnt = wpool.tile([128, 1], F32, tag="cnt", bufs=2)
                        nc.vector.tensor_scalar(cmp2[:], ps_sel[:],
                                                tau[:, 0:1], 0.0, cmp_op,
                                                OP.add, accum_out=cnt[:])
                        sidx = wpool.tile([128, 1], F32, tag="sidx", bufs=2)
                        nc.vector.tensor_scalar(sidx[:], bcnt[:], 128.0,
                                                cnt[:, 0:1], OP.mult, OP.add)
                        idr_f = wpool.tile([128, 1], F32, tag="idrf", bufs=2)
                        nc.vector.tensor_scalar_min(idr_f[:], sidx[:],
                                                    float(S))
                        nc.vector.tensor_copy(idxR[b][kk][:, j:j + 1],
                                              idr_f[:])
                        idx_f = wpool.tile([128, 1], F32, tag="idxf", bufs=2)
                        nc.vector.tensor_scalar_min(idx_f[:], sidx[:],
                                                    float(S - 1))
                        nc.vector.tensor_copy(idxX[b][kk][:, j:j + 1],
                                              idx_f[:])
                        zc = wpool.tile([128, 1], F32, tag="zc", bufs=2)
                        nc.vector.tensor_scalar(zc[:], cnt[:], 0.0, None,
                                                OP.is_equal)
                        blkf = wpool.tile([128, 1], F32, tag="blkf", bufs=2)
                        nc.vector.tensor_sub(blkf[:], bcnt[:], zc[:])
                        nc.vector.tensor_copy(blki[b][kk][:, j:j + 1],
                                              blkf[:])

                        # gathers
                        nc.gpsimd.indirect_dma_start(
                            out=cprev[b][kk][:, j:j + 1], out_offset=None,
                            in_=csum_dram[b][:],
                            in_offset=bass.IndirectOffsetOnAxis(
                                ap=idxR[b][kk][:, j:j + 1], axis=0))
                        nc.gpsimd.indirect_dma_start(
                            out=gr[b][kk][:, j, :], out_offset=None,
                            in_=R_dram[b][:],
                            in_offset=bass.IndirectOffsetOnAxis(
                                ap=idxR[b][kk][:, j:j + 1], axis=0))
                        nc.gpsimd.indirect_dma_start(
                            out=gr[b][kk][:, j, :], out_offset=None,
                            in_=offs_dram[b][:],
                            in_offset=bass.IndirectOffsetOnAxis(
                                ap=blki[b][kk][:, j:j + 1], axis=0),
                            bounds_check=31, oob_is_err=False,
                            compute_op=OP.add)
                        nc.gpsimd.indirect_dma_start(
                            out=gx[b][kk][:, j, :], out_offset=None,
                            in_=x_flat,
                            in_offset=bass.IndirectOffsetOnAxis(
                                ap=idxX[b][kk][:, j:j + 1], axis=0),
                            element_offset=b * S * C)

            # ================= combine & write out =========================
            def combine(b):
                scale_c = cols[b][:, 3:4]
                negscale_c = cols[b][:, 6:7]
                L_c = cols[b][:, 7:8]
                for j in range(NT):
                    tcol = wpool.tile([128, 1], F32, tag="tcol", bufs=2)
                    nc.vector.tensor_scalar_add(tcol[:], iota_col[:],
                                                float(128 * j))
                    valid = wpool.tile([128, 1], F32, tag="valid", bufs=2)
                    nc.vector.tensor_scalar(valid[:], tcol[:], L_c, None,
                                            OP.is_lt)
                    c1 = wpool.tile([128, 1], F32, tag="c1", bufs=2)
                    nc.vector.tensor_scalar(c1[:], cprev[b][0][:, j:j + 1],
                                            scale_c, tcol[:, 0:1], OP.mult,
                                            OP.subtract)
                    nc.vector.tensor_mul(c1[:], c1[:], valid[:])
                    t1col = wpool.tile([128, 1], F32, tag="t1col", bufs=2)
                    nc.vector.tensor_scalar_add(t1col[:], tcol[:], 1.0)
                    c2 = wpool.tile([128, 1], F32, tag="c2", bufs=2)
                    nc.vector.tensor_scalar(c2[:], cprev[b][1][:, j:j + 1],
                                            negscale_c, t1col[:, 0:1],
                                            OP.mult, OP.add)
                    nc.vector.tensor_mul(c2[:], c2[:], valid[:])
                    sv = wpool.tile([128, 1], F32, tag="sv", bufs=2)
                    nc.vector.tensor_mul(sv[:], scale_c, valid[:])

                    d = wpool.tile([128, C], F32, tag="d", bufs=2)
                    nc.vector.tensor_sub(d[:], gr[b][1][:, j, :],
                                         gr[b][0][:, j, :])
                    t0 = wpool.tile([128, C], F32, tag="t0", bufs=2)
                    nc.scalar.activation(t0[:], gx[b][0][:, j, :], AF.Copy,
                                         scale=c1[:, 0:1])
                    t1 = wpool.tile([128, C], F32, tag="t1", bufs=2)
                    nc.vector.scalar_tensor_tensor(t1[:], gx[b][1][:, j, :],
                                                   c2[:, 0:1], t0[:], OP.mult,
                                                   OP.add)
                    ot = wpool.tile([128, C], F32, tag="ot", bufs=2)
                    nc.vector.scalar_tensor_tensor(ot[:], d[:], sv[:, 0:1],
                                                   t1[:], OP.mult, OP.add)
                    nc.sync.dma_start(out_d[b, 128 * j:128 * (j + 1), :],
                                      ot[:])

            # ================= emission order ==============================
            KSTAGE = int(os.environ.get("KSTAGE", "9"))
            phaseAB(0)
            if KSTAGE >= 1:
                csum_scale(0)
            if KSTAGE >= 2:
                rscan(0)
            if KSTAGE >= 3:
                phaseAB(1)
            if KSTAGE >= 4:
                search(0)
            if KSTAGE >= 5:
                combine(0)
            if KSTAGE >= 6:
                csum_scale(1)
                rscan(1)
                search(1)
                combine(1)

    nc.compile()
    return nc


_prog_cache = {}


def _get_prog(flags=(True, True, True, True)):
    if flags not in _prog_cache:
        _prog_cache[flags] = build_program(*flags)
    return _prog_cache[flags]


def make_in_maps(inputs):
    """Host-side preprocessing: shard, cast x to fp16, pre-transpose conv_w."""
    x = np.asarray(inputs["x"], np.float32)
    pad = np.asarray(inputs["encoder_padding_mask"]).astype(np.uint8)
    tl = np.asarray(inputs["target_lengths"]).astype(np.int32)
    conv_w = np.asarray(inputs["conv_w"], np.float32)
    conv_b = np.asarray(inputs["conv_b"], np.float32)
    ln_g = np.asarray(inputs["ln_g"], np.float32)
    ln_b = np.asarray(inputs["ln_b"], np.float32)
    proj_w = np.asarray(inputs["proj_w"], np.float32)
    proj_b = np.asarray(inputs["proj_b"], np.float32)

    g1b0 = bool(np.all(ln_g == 1.0) and np.all(ln_b == 0.0))
    cb0 = bool(np.all(conv_b == 0.0))
    pb0 = bool(np.all(proj_b == 0.0))
    pad0 = bool(not pad.any())
    flags = (g1b0, cb0, pb0, pad0)

    x16 = x.astype(np.float16)
    w16 = conv_w.astype(np.float16)
    wt = np.empty((128, 12, C), np.float16)
    for k in range(3):
        for q in range(4):
            wt[:, k * 4 + q, :] = w16[:, 128 * q:128 * (q + 1), k].T
    pw_rep = np.ascontiguousarray(
        np.broadcast_to(proj_w[:, 0][None, :], (128, C)).astype(np.float32))
    ut128 = np.triu(np.ones((128, 128), np.float16), 0)

    in_maps = []
    for core in range(NCORES):
        lo, hi = core * BL, (core + 1) * BL
        m = {
            "x": np.ascontiguousarray(x16[lo:hi]),
            "wt": wt,
            "pw_rep": pw_rep,
            "ut128": ut128,
            "target_lengths": np.ascontiguousarray(tl[lo:hi]),
        }
        if not cb0:
            m["convb16"] = conv_b.astype(np.float16)[None, :]
        if not g1b0:
            m["g_rep"] = np.ascontiguousarray(
                np.broadcast_to(ln_g[None, :], (128, C)).astype(np.float32))
            m["b_rep"] = np.ascontiguousarray(
                np.broadcast_to(ln_b[None, :], (128, C)).astype(np.float32))
        if not pb0:
            m["pb_col"] = np.full((128, 1), float(proj_b[0]), np.float32)
        if not pad0:
            m["encoder_padding_mask"] = np.ascontiguousarray(pad[lo:hi])
        in_maps.append(m)
    return flags, in_maps


def kernel(**inputs):
    flags, in_maps = make_in_maps(inputs)
    nc = _get_prog(flags)
    res = run_bass_kernel_spmd(nc, in_maps, core_ids=list(range(NCORES)))
    out = np.concatenate([res.results[c]["out"] for c in range(NCORES)],
                         axis=0)
    return out.astype(np.float32)


if __name__ == "__main__":
    import reference as ref
    import jax
    jax.config.update("jax_platforms", "cpu")
    inputs = ref.setup_inputs()
    actual = kernel(**{k: np.asarray(v) for k, v in inputs.items()})
    print("kernel output", actual.shape, actual.dtype)
